# revision 11
# baseline (speedup 1.0000x reference)
"""Kernel herding (greedy thinning) on Trainium2 via Bass/Tile.

Reference semantics:
  K[i,j] = exp(-0.5*(||xi||^2 + ||xj||^2 - 2 xi.xj))   (RBF, lengthscale 1)
  k0_mean = row-mean of K;  obj = 1 - 2*k0_mean;  m-1 greedy steps of
  obj = (obj + 2*K[idx]) - 2*k0_mean, argmin with first-index tie-breaking.

Device implementation (single NeuronCore, full problem on-chip):
  Phase 1 (construction): S = 2*K materialized in device DRAM (bf16) via
    bf16 matmuls (X.T tiles) + a rank-1 matmul folding -0.5*||xj||^2 into
    PSUM + ScalarE Exp with per-partition bias (-0.5*||xi||^2 + ln 2) and
    fused accum_out row-sums.  The Gram diagonal from this path is
    2*exp(eps) != 2 (bf16 roundoff); row-sums are corrected to the exact
    2*K[i,i] = 2.  The slab keeps the approximate diagonal: its only role
    is pushing picked rows out of contention (margin ~1.2 vs max drift
    2*m/N = 0.0625).
  Phase 2 (scan): maintain mobj = 2 - obj (argmax mobj == argmin obj).
    Per step: fetch row S[j*] via register-offset DMA, mobj -= row,
    mobj += d (d = 2*k0_mean), argmax via DVE max8 + max_index
    (first-occurrence == reference first-index tie-break), cross-partition
    combine via two PE transposes + small max8/max_index, register loads
    to form the next row address.

obj layout: [128 partitions, FB free], j = p*FB + f (FB = N/128), so a
DRAM row (N contiguous) lands as [128, FB] partition-major.
"""

import os

import numpy as np

N, D, M = 16384, 128, 512
P = 128

_LN2 = 0.6931471805599453


def _host_kernel(x, m):
    x = np.ascontiguousarray(x, dtype=np.float32)
    sq = np.sum(x * x, axis=1, dtype=np.float32)
    n = x.shape[0]
    Kmat = np.empty((n, n), np.float32)
    for i0 in range(0, n, 2048):
        g = x[i0:i0 + 2048] @ x.T
        d2 = (sq[i0:i0 + 2048, None] + sq[None, :]) - np.float32(2.0) * g
        Kmat[i0:i0 + 2048] = np.exp(d2 * np.float32(-0.5), dtype=np.float32)
    k0m = (Kmat.sum(axis=1, dtype=np.float32) / np.float32(n)).astype(np.float32)
    two_k0m = np.float32(2.0) * k0m
    obj = (np.float32(1.0) - two_k0m).astype(np.float32)
    idx = int(np.argmin(obj))
    out = np.empty(m, dtype=np.int32)
    out[0] = idx
    for t in range(1, m):
        obj = ((obj + np.float32(2.0) * Kmat[idx]) - two_k0m).astype(np.float32)
        idx = int(np.argmin(obj))
        out[t] = idx
    return out


def build_nc(n=N, m=M, slab_dtype="bfloat16"):
    """Build the Bass program for problem size (n, D) and m picks."""
    os.environ.setdefault("NEURON_SCRATCHPAD_PAGE_SIZE", "1024")
    import concourse.bass as bass
    import concourse.bacc as bacc
    import concourse.tile as tile
    import concourse.mybir as mybir

    fp32 = mybir.dt.float32
    bf16 = mybir.dt.bfloat16
    i32 = mybir.dt.int32
    u32 = mybir.dt.uint32
    sdt = getattr(mybir.dt, slab_dtype)
    SP = mybir.EngineType.SP

    FB = n // P                 # obj free size per partition
    NCH = n // P                # i-chunks of 128 rows
    GRP = min(2048, n)          # j-group width (one wide PSUM tile)
    NG = n // GRP               # j-groups
    JT = min(512, GRP)          # matmul free tile
    NJT = GRP // JT

    nc = bacc.Bacc(None, target_bir_lowering=False, debug=False)
    x_ext = nc.dram_tensor("x", [n, D], fp32, kind="ExternalInput")
    out_ext = nc.dram_tensor("out", [1, m], i32, kind="ExternalOutput")
    slab = nc.dram_tensor("slab", [n, n], sdt)          # internal DRAM
    slabv = slab.reshape([n * P, FB])                   # row j -> P rows of FB

    with tile.TileContext(nc) as tc:
        with tc.tile_pool(name="persist", bufs=1) as pers, \
             tc.tile_pool(name="work", bufs=3) as work:

            # ------------- persistent tiles -------------
            xT = pers.tile([P, n], bf16)            # [d, i] for matmuls
            ident = pers.tile([P, P], fp32)
            identb = pers.tile([P, P], bf16)
            ones1 = pers.tile([1, P], bf16)
            sqjm = pers.tile([1, n], bf16)          # -0.5*sq in j-row layout
            biasln = pers.tile([P, NCH], fp32)      # -0.5*sq + ln2, [p, b]
            acc = pers.tile([P, NG], fp32)          # per-group 2K row sums
            rs = pers.tile([P, NCH], fp32)          # chunk row sums, [i%128, i//128]
            mobj = pers.tile([P, FB], fp32)
            dtile = pers.tile([P, FB], fp32)
            outsb = pers.tile([1, m], i32)

            nc.vector.memset(ones1, 1.0)
            it0 = work.tile([P, P], i32, tag="it0")
            nc.gpsimd.iota(it0, pattern=[[1, P]], base=0, channel_multiplier=-1)
            nc.vector.tensor_scalar(ident, it0, 0, None, mybir.AluOpType.is_equal)
            nc.vector.tensor_copy(identb, ident)
            # negid: -1e30 on the diagonal (forces exp->0 there); twoidb: +2 diag
            negid = pers.tile([P, P], fp32)
            nc.vector.tensor_scalar(negid, ident, -1.0e30, None,
                                    mybir.AluOpType.mult)
            twoid = pers.tile([P, P], fp32)
            nc.vector.tensor_scalar(twoid, ident, 2.0, None, mybir.AluOpType.mult)
            twoidb = pers.tile([P, P], bf16)
            nc.vector.tensor_copy(twoidb, twoid)

            # ------------- prologue -------------
            with tc.tile_pool(name="prol", bufs=3) as prol, \
                 tc.tile_pool(name="prol1", bufs=1) as prol1, \
                 tc.tile_pool(name="pp1", bufs=2, space="PSUM") as pp1:
                xbf = prol1.tile([P, NCH * D], bf16)
                sq = prol1.tile([P, NCH], fp32)
                for b in range(NCH):
                    xc = prol.tile([P, D], fp32, tag="xc")
                    nc.sync.dma_start(xc, x_ext[b * P:(b + 1) * P, :])
                    nc.vector.tensor_copy(xbf[:, b * D:(b + 1) * D], xc)
                    sqdump = prol.tile([P, D], fp32, tag="sqdump")
                    nc.scalar.activation(sqdump, xc,
                                         mybir.ActivationFunctionType.Square,
                                         accum_out=sq[:, b:b + 1])
                sqm = prol1.tile([P, NCH], fp32)
                nc.vector.tensor_scalar(sqm, sq, -0.5, None, mybir.AluOpType.mult)
                nc.vector.tensor_scalar(biasln, sqm, _LN2, None,
                                        mybir.AluOpType.add)

                # xT blocks via PE transpose
                for b in range(NCH):
                    pt = pp1.tile([P, P], bf16, tag="pt")
                    nc.tensor.transpose(pt, xbf[:, b * D:(b + 1) * D], identb)
                    if b % 2 == 0:
                        nc.scalar.copy(xT[:, b * P:(b + 1) * P], pt)
                    else:
                        nc.vector.tensor_copy(xT[:, b * P:(b + 1) * P], pt)

                # sqjm[0, j] = -0.5*sq[j], j = b*128+p: transpose then flatten
                pt2 = pp1.tile([NCH, P], fp32, tag="pt2")
                nc.tensor.transpose(pt2, sqm, ident)
                sqmTb = prol.tile([NCH, P], bf16)
                nc.vector.tensor_copy(sqmTb, pt2)
                nc.sync.dma_start(sqjm[0:1, :], sqmTb[:, :])

            # ------------- construction -------------
            with tc.tile_pool(name="cpsum", bufs=2, space="PSUM") as cp:
                for b in range(NCH):
                    lhsT = xT[:, b * P:(b + 1) * P]
                    for g in range(NG):
                        pg = cp.tile([P, GRP], fp32, tag="pg")
                        for jt in range(NJT):
                            j0 = g * GRP + jt * JT
                            nc.tensor.matmul(pg[:, jt * JT:(jt + 1) * JT],
                                             lhsT, xT[:, j0:j0 + JT],
                                             start=True, stop=False)
                            nc.tensor.matmul(pg[:, jt * JT:(jt + 1) * JT],
                                             ones1, sqjm[0:1, j0:j0 + JT],
                                             start=False, stop=True)
                        if g == (b * P) // GRP:
                            # force exp->0 exactly on the diagonal (uniform
                            # across rows; the exact +2 is re-added below)
                            o = (b * P) % GRP
                            nc.vector.tensor_tensor(
                                out=pg[:, o:o + P], in0=pg[:, o:o + P],
                                in1=negid, op=mybir.AluOpType.add)
                        ssb = work.tile([P, GRP], sdt, tag="ssb")
                        nc.scalar.activation(ssb, pg,
                                             mybir.ActivationFunctionType.Exp,
                                             bias=biasln[:, b:b + 1], scale=1.0,
                                             accum_out=acc[:, g:g + 1])
                        if g == (b * P) // GRP:
                            # exact slab diagonal 2*K[i,i] = 2
                            o = (b * P) % GRP
                            nc.vector.tensor_tensor(
                                out=ssb[:, o:o + P], in0=ssb[:, o:o + P],
                                in1=twoidb, op=mybir.AluOpType.add)
                        nc.sync.dma_start(
                            slab[b * P:(b + 1) * P, g * GRP:(g + 1) * GRP], ssb)
                    nc.vector.tensor_reduce(out=rs[:, b:b + 1], in_=acc[:, 0:NG],
                                            op=mybir.AluOpType.add,
                                            axis=mybir.AxisListType.X)

            # ------------- k0_mean -> dtile, mobj init -------------
            with tc.tile_pool(name="ppost", bufs=2, space="PSUM") as pp2, \
                 tc.tile_pool(name="post", bufs=1) as post:
                ptd = pp2.tile([NCH, P], fp32, tag="ptd")
                nc.tensor.transpose(ptd, rs, ident)
                rsT = post.tile([NCH, P], fp32)      # rsT[b, p] = rowsum(i=b*128+p)
                nc.scalar.copy(rsT, ptd)
                # linear i-order -> [P, FB] (j = p*FB + f) via SBUF->SBUF DMA
                rsL = post.tile([P, FB], fp32)
                nc.sync.dma_start(rsL[:, :], rsT[:, :])
                nc.vector.tensor_scalar(dtile, rsL, 2.0, 1.0 / n,
                                        mybir.AluOpType.add,
                                        mybir.AluOpType.mult)
                # mobj = 2 - obj0 = 1 + d
                nc.vector.tensor_scalar(mobj, dtile, 1.0, None,
                                        mybir.AluOpType.add)

            # ------------- scan -------------
            with tc.tile_pool(name="spsum", bufs=2, space="PSUM") as sp:
                for t in range(m):
                    m8 = work.tile([P, 8], fp32, tag="m8")
                    i8 = work.tile([P, 8], u32, tag="i8")
                    nc.vector.max(m8, mobj)
                    nc.vector.max_index(i8, m8, mobj)
                    i8f = work.tile([P, 1], fp32, tag="i8f")
                    nc.vector.tensor_copy(i8f, i8[:, 0:1])
                    mrow = sp.tile([1, P], fp32, tag="mrow")
                    frow = sp.tile([1, P], fp32, tag="frow")
                    nc.tensor.transpose(mrow, m8[:, 0:1], ident)
                    nc.tensor.transpose(frow, i8f, ident)
                    fu32 = work.tile([1, P], u32, tag="fu32")
                    nc.vector.tensor_copy(fu32, frow)
                    mm8 = work.tile([1, 8], fp32, tag="mm8")
                    pp8 = work.tile([1, 8], u32, tag="pp8")
                    nc.vector.max(mm8, mrow)
                    nc.vector.max_index(pp8, mm8, mrow)
                    pstar = nc.values_load(pp8[0:1, 0:1], engines=[SP],
                                           min_val=0, max_val=P - 1,
                                           skip_runtime_bounds_check=True)
                    fstar = nc.values_load(fu32[0:1, bass.ds(pstar, 1)],
                                           engines=[SP],
                                           min_val=0, max_val=FB - 1,
                                           skip_runtime_bounds_check=True)
                    jstar = pstar * FB + fstar
                    nc.sync.reg_save(outsb[0:1, t:t + 1], jstar)
                    if t + 1 < m:
                        rowbuf = work.tile([P, FB], sdt, tag="rowbuf")
                        nc.sync.dma_start(rowbuf,
                                          slabv[bass.ds(jstar * P, P), :])
                        nc.vector.tensor_tensor(out=mobj, in0=mobj, in1=rowbuf,
                                                op=mybir.AluOpType.subtract)
                        nc.vector.tensor_tensor(out=mobj, in0=mobj, in1=dtile,
                                                op=mybir.AluOpType.add)
            nc.sync.dma_start(out_ext[:, :], outsb)

    nc.compile()
    return nc


_CACHE = {}


def _get_nc(n, m):
    key = (n, m)
    if key not in _CACHE:
        _CACHE[key] = build_nc(n, m)
    return _CACHE[key]


def _device_kernel(x, m):
    nc = _get_nc(x.shape[0], m)
    from concourse.bass_utils import run_bass_kernel_spmd
    res = run_bass_kernel_spmd(nc, [{"x": x}], core_ids=[0], trace=False)
    return np.asarray(res.results[0]["out"][0], dtype=np.int32)


def kernel(x, m):
    m = int(m)
    x = np.ascontiguousarray(np.asarray(x, dtype=np.float32))
    assert x.shape == (N, D), x.shape
    if os.environ.get("HERD_HOST_ONLY", "0") == "1":
        return _host_kernel(x, m)
    try:
        return _device_kernel(x, m)
    except Exception as e:
        import traceback
        traceback.print_exc()
        print(f"[kernel.py] device path failed ({type(e).__name__}); "
              f"falling back to host")
        return _host_kernel(x, m)


if __name__ == "__main__":
    import jax
    x = np.asarray(jax.random.normal(jax.random.key(0), (N, D)),
                   dtype=np.float32)
    out = kernel(x, M)
    print(out[:16])
    print("arange match:", bool(np.array_equal(out, np.arange(M))))
